# revision 18
# baseline (speedup 1.0000x reference)
"""Trainium2 Bass kernel for a dense recurrent scan (nn_CXBPU_55611236549128).

Math (per timestep t, K=4 microsteps):
    inj  = x_t @ W_in.T + b_in                  scattered into sensory_indices
    h    = relu(h @ W_rec.T + scatter(inj))     microstep 0
    h    = relu(h @ W_rec.T)                    microsteps 1..K-1
    out_t = h[:, output_indices] @ W_out.T + b_out

Sharding: data-parallel over batch, 8 rows per core, W_rec replicated.

Per-core design (feature-major "hT" layout [128 partitions, 16 chunks x 8 batch]):
  - W_rec.T resident in SBUF, streamed as the *moving* matmul operand every
    microstep (h-stationary keeps the weight transit on the fast streaming
    port instead of the 1.2 GHz LDWEIGHTS port).
  - Precision: W = W1 + W2 with both halves fp16 (exact 22-bit split; fp16
    subnormals are exact on the PE), h quantized to fp16 once per microstep
    by the relu write. Two fp16 passes accumulate in fp32 PSUM. End-to-end
    error vs fp32 reference ~4e-4 scale-relative absmax (the recurrence is
    contractive, spectral radius 0.9, so per-step quantization damps).
  - 4 k-tiles run concurrently in 4 PE column groups (tile_position=(0,32j)),
    issued column-group-innermost so the streams overlap.
  - A "transpose-sum" matmul against a 0/1 selector (i128) folds the 4
    partition groups back into feature-major hT for the next microstep
    (exact: fp16 values pass through fp32 PSUM untouched).
  - Injection is added as a host-precomputed dense tile already in hT layout.
  - Readout: 16 tiny matmuls vs scatter-expanded W_out (wsel).
"""

import os
from contextlib import ExitStack

import numpy as np

N = 2048
B = 64
T = 128
NCORES = 8
BPC = B // NCORES  # 8 batch rows per core
NCHUNK = N // 128  # 16

_CACHE = {}

# 'fp16x2' = two-pass fp16 split (fast), 'fp32' = exact fp32 (4-pass, slow)
MODE = os.environ.get("KERNEL_MM_MODE", "fp16x2")


def _build_nc(n_steps, mode=MODE):
    import concourse.bass as bass
    import concourse.mybir as mybir
    import concourse.tile as tile
    from concourse import bacc

    f32 = mybir.dt.float32
    f16 = mybir.dt.float16
    fmm = f16 if mode.startswith("fp16") else f32
    npass = 2 if mode == "fp16x2" else 1
    nc = bacc.Bacc(trn_type="TRN2")

    wt_d = nc.dram_tensor("wt", [npass * N, N], fmm, kind="ExternalInput")
    injd_d = nc.dram_tensor("injd", [n_steps, 128, 128], f32, kind="ExternalInput")
    wsel_d = nc.dram_tensor("wsel", [128, 2 * NCHUNK], fmm, kind="ExternalInput")
    i128_d = nc.dram_tensor("i128", [128, BPC], fmm, kind="ExternalInput")
    out_d = nc.dram_tensor("out", [2, n_steps * BPC], f32, kind="ExternalOutput")

    NSLAB = npass * NCHUNK

    with tile.TileContext(nc) as tc, ExitStack() as ctx:
        const = ctx.enter_context(tc.tile_pool(name="const", bufs=1))
        hpool = ctx.enter_context(tc.tile_pool(name="h", bufs=2))
        epool = ctx.enter_context(tc.tile_pool(name="evac", bufs=2))
        ipool = ctx.enter_context(tc.tile_pool(name="injd", bufs=2))
        ppool = ctx.enter_context(tc.tile_pool(name="psum", bufs=1, space="PSUM"))
        tpool = ctx.enter_context(tc.tile_pool(name="psumT", bufs=3, space="PSUM"))
        rpool = ctx.enter_context(tc.tile_pool(name="psumR", bufs=1, space="PSUM"))

        # resident W^T slabs: slab u = pass*16 + k-tile at cols [u*2048, ...).
        # Spread the 16 MB load across both HWDGE families + SWDGE.
        wt = const.tile([128, NSLAB * N], fmm)
        for u in range(NSLAB):
            eng = (nc.sync, nc.scalar, nc.gpsimd)[u % 3]
            eng.dma_start(wt[:, u * N : (u + 1) * N], wt_d[u * 128 : (u + 1) * 128, :])
        i128 = const.tile([128, BPC], fmm)
        nc.sync.dma_start(i128[:], i128_d[:])
        wsel = const.tile([128, 2 * NCHUNK], fmm)
        nc.sync.dma_start(wsel[:], wsel_d[:])
        outst = const.tile([2, n_steps * BPC], f32)

        psum = ppool.tile([128, N], f32)
        nc.vector.memset(psum[:], 0.0)

        hT = hpool.tile([128, NCHUNK * BPC], fmm)
        nc.vector.memset(hT[:], 0.0)

        tc.strict_bb_all_engine_barrier()

        for t in range(n_steps):
            injd = ipool.tile([128, 128], f32)
            nc.sync.dma_start(injd[:], injd_d[t])
            for s in range(4):
                # ---- main matmuls: psum[32j+b, n] += sum_k h[b,k] Wrec[n,k]
                # Bank-outer so bank n finishes early; its PSUM->SBUF evac and
                # transpose-sum matmuls then hide behind bank n+1's matmuls.
                # Within a bank, col-group j handles k-tiles {4r+j}; j
                # innermost so the 4 column-group streams overlap.
                evac = epool.tile([128, N], fmm)
                psumT = tpool.tile([128, NCHUNK * BPC], f32)

                def main_bank(n):
                    for r in range(4):
                        for p in range(npass):
                            for j in range(4):
                                kk = 4 * r + j
                                u = p * NCHUNK + kk
                                nc.tensor.matmul(
                                    psum[32 * j : 32 * j + BPC, 512 * n : 512 * (n + 1)],
                                    lhsT=hT[:, kk * BPC : (kk + 1) * BPC],
                                    rhs=wt[:, u * N + 512 * n : u * N + 512 * (n + 1)],
                                    start=(r == 0 and p == 0),
                                    stop=(r == 3 and p == npass - 1),
                                    tile_position=(0, 32 * j),
                                )

                def evac_bank(n):
                    # ACT copies cost ~2 us vs ~0.7 us on DVE; with 1-pass main
                    # matmuls the banks are too short to hide ACT, so keep all
                    # evacs on DVE there and alternate engines only for 2-pass.
                    if npass == 1 or n % 2 == 0:
                        nc.vector.tensor_copy(
                            evac[:, 512 * n : 512 * (n + 1)], psum[:, 512 * n : 512 * (n + 1)]
                        )
                    else:
                        nc.scalar.copy(
                            evac[:, 512 * n : 512 * (n + 1)], psum[:, 512 * n : 512 * (n + 1)]
                        )

                def tmm_bank(n):
                    # transpose-sum: psumT[m, c*8+b] = sum_j psum[32j+b, c*128+m]
                    for c in range(4 * n, 4 * n + 4):
                        nc.tensor.matmul(
                            psumT[:, c * BPC : (c + 1) * BPC],
                            lhsT=evac[:, c * 128 : (c + 1) * 128],
                            rhs=i128[:],
                            start=True,
                            stop=True,
                        )

                hT_new = hpool.tile([128, NCHUNK * BPC], fmm)

                def relu_bank(n):
                    # chunks 4n..4n+3 -> hT cols [32n, 32n+32); round r of the
                    # next microstep depends only on relu_bank(r).
                    cs = slice(32 * n, 32 * n + 32)
                    if s == 0:
                        nc.vector.tensor_add(hT_new[:, cs], psumT[:, cs], injd[:, cs])
                        nc.vector.tensor_relu(hT_new[:, cs], hT_new[:, cs])
                    else:
                        nc.vector.tensor_relu(hT_new[:, cs], psumT[:, cs])

                main_bank(0)
                evac_bank(0)
                main_bank(1)
                evac_bank(1)
                tmm_bank(0)
                relu_bank(0)
                main_bank(2)
                evac_bank(2)
                tmm_bank(1)
                relu_bank(1)
                main_bank(3)
                evac_bank(3)
                tmm_bank(2)
                relu_bank(2)
                tmm_bank(3)
                relu_bank(3)
                hT = hT_new
            # ---- readout for timestep t from final hT
            pr = rpool.tile([2, BPC], f32)
            for c in range(NCHUNK):
                nc.tensor.matmul(
                    pr[:],
                    lhsT=wsel[:, c * 2 : (c + 1) * 2],
                    rhs=hT[:, c * BPC : (c + 1) * BPC],
                    start=(c == 0),
                    stop=(c == NCHUNK - 1),
                )
            nc.vector.tensor_copy(outst[:, t * BPC : (t + 1) * BPC], pr[:])

        nc.sync.dma_start(out_d[:], outst[:])
    nc.compile()
    return nc


def _prep_inputs(inputs, W_rec, W_in, b_in, W_out, sensory_indices, output_indices,
                 n_steps, mode=MODE):
    inputs = np.asarray(inputs, np.float32)
    W_rec = np.asarray(W_rec, np.float32)
    W_in = np.asarray(W_in, np.float32)
    b_in = np.asarray(b_in, np.float32)
    W_out = np.asarray(W_out, np.float32)
    sens = np.asarray(sensory_indices).astype(np.int64)
    oidx = np.asarray(output_indices).astype(np.int64)

    wtf = np.ascontiguousarray(W_rec.T)
    wsel_full = np.zeros((2, N), np.float32)
    np.add.at(wsel_full, (slice(None), oidx), W_out)
    wself = wsel_full.reshape(2, NCHUNK, 128).transpose(2, 1, 0).reshape(128, 2 * NCHUNK)

    if mode.startswith("fp16"):
        w1 = wtf.astype(np.float16)
        if mode == "fp16x2":
            w2 = (wtf - w1.astype(np.float32)).astype(np.float16)
            wt = np.ascontiguousarray(np.concatenate([w1, w2], axis=0))
        else:
            wt = np.ascontiguousarray(w1)
        wsel = np.ascontiguousarray(wself.astype(np.float16))
        i128 = (np.arange(128)[:, None] % 32 == np.arange(BPC)[None, :]).astype(np.float16)
    else:
        wt = wtf
        wsel = np.ascontiguousarray(wself)
        i128 = (np.arange(128)[:, None] % 32 == np.arange(BPC)[None, :]).astype(np.float32)

    # dense injection in hT layout, per core
    inj_all = inputs[:, :n_steps, :] @ W_in.T + b_in  # [B, T, 256]
    inj_dense = np.zeros((B, n_steps, N), np.float32)
    np.add.at(inj_dense, (slice(None), slice(None), sens), inj_all)
    injd_cores = []
    for g in range(NCORES):
        a = inj_dense[g * BPC : (g + 1) * BPC]  # [8, T, 2048]
        a = a.reshape(BPC, n_steps, NCHUNK, 128).transpose(1, 3, 2, 0)
        injd_cores.append(np.ascontiguousarray(a.reshape(n_steps, 128, NCHUNK * BPC)))

    return wt, injd_cores, wsel, i128


def _run(inputs, W_rec, W_in, b_in, W_out, b_out, sensory_indices, output_indices,
         K, n_steps=T, trace=False, mode=MODE):
    from concourse.bass_utils import run_bass_kernel_spmd

    assert int(K) == 4
    wt, injd_cores, wsel, i128 = _prep_inputs(
        inputs, W_rec, W_in, b_in, W_out, sensory_indices, output_indices,
        n_steps, mode)

    key = (n_steps, mode)
    if key not in _CACHE:
        _CACHE[key] = _build_nc(n_steps, mode)
    nc = _CACHE[key]

    in_maps = [
        {"wt": wt, "injd": injd_cores[g], "wsel": wsel, "i128": i128}
        for g in range(NCORES)
    ]
    res = run_bass_kernel_spmd(nc, in_maps, list(range(NCORES)), trace=trace)

    b_out = np.asarray(b_out, np.float32)
    outs = []
    for g in range(NCORES):
        r = np.asarray(res.results[g]["out"])  # [2, T*8]
        outs.append(r.reshape(2, n_steps, BPC).transpose(2, 1, 0))  # [8, T, 2]
    full = np.concatenate(outs, axis=0) + b_out  # [B, T, 2]
    return np.ascontiguousarray(full.astype(np.float32)), res


def kernel(**inputs):
    out, _ = _run(
        inputs["inputs"], inputs["W_rec"], inputs["W_in"], inputs["b_in"],
        inputs["W_out"], inputs["b_out"], inputs["sensory_indices"],
        inputs["output_indices"], inputs["K"],
    )
    return out


# revision 22
# speedup vs baseline: 1.0003x; 1.0003x over previous
"""Trainium2 Bass kernel for a dense recurrent scan (nn_CXBPU_55611236549128).

Math (per timestep t, K=4 microsteps):
    inj  = x_t @ W_in.T + b_in                  scattered into sensory_indices
    h    = relu(h @ W_rec.T + scatter(inj))     microstep 0
    h    = relu(h @ W_rec.T)                    microsteps 1..K-1
    out_t = h[:, output_indices] @ W_out.T + b_out

Sharding: data-parallel over batch, 8 rows per core, W_rec replicated.

Per-core design (feature-major "hT" layout [128 partitions, 16 chunks x 8 batch]):
  - W_rec.T resident in SBUF, streamed as the *moving* matmul operand every
    microstep (h-stationary keeps the weight transit on the fast streaming
    port instead of the 1.2 GHz LDWEIGHTS port).
  - Precision: W = W1 + W2 with both halves fp16 (exact 22-bit split; fp16
    subnormals are exact on the PE), h quantized to fp16 once per microstep
    by the relu write. Two fp16 passes accumulate in fp32 PSUM. End-to-end
    error vs fp32 reference ~4e-4 scale-relative absmax (the recurrence is
    contractive, spectral radius 0.9, so per-step quantization damps).
  - 4 k-tiles run concurrently in 4 PE column groups (tile_position=(0,32j)),
    issued column-group-innermost so the streams overlap.
  - A "transpose-sum" matmul against a 0/1 selector (i128) folds the 4
    partition groups back into feature-major hT for the next microstep
    (exact: fp16 values pass through fp32 PSUM untouched).
  - Injection is added as a host-precomputed dense tile already in hT layout.
  - Readout: 16 tiny matmuls vs scatter-expanded W_out (wsel).
"""

import os
from contextlib import ExitStack

import numpy as np

N = 2048
B = 64
T = 128
NCORES = 8
BPC = B // NCORES  # 8 batch rows per core
NCHUNK = N // 128  # 16

_CACHE = {}

# 'fp16x2' = two-pass fp16 split (fast), 'fp32' = exact fp32 (4-pass, slow)
MODE = os.environ.get("KERNEL_MM_MODE", "fp16x2")


def _build_nc(n_steps, mode=MODE):
    import concourse.bass as bass
    import concourse.mybir as mybir
    import concourse.tile as tile
    from concourse import bacc

    f32 = mybir.dt.float32
    f16 = mybir.dt.float16
    fmm = f16 if mode.startswith("fp16") else f32
    npass = 2 if mode == "fp16x2" else 1
    nc = bacc.Bacc(trn_type="TRN2")

    wt_d = nc.dram_tensor("wt", [npass * N, N], fmm, kind="ExternalInput")
    injd_d = nc.dram_tensor("injd", [n_steps, 128, 128], f32, kind="ExternalInput")
    wsel_d = nc.dram_tensor("wsel", [128, 2 * NCHUNK], fmm, kind="ExternalInput")
    i128_d = nc.dram_tensor("i128", [128, BPC], fmm, kind="ExternalInput")
    out_d = nc.dram_tensor("out", [2, n_steps * BPC], f32, kind="ExternalOutput")

    NSLAB = npass * NCHUNK

    with tile.TileContext(nc) as tc, ExitStack() as ctx:
        const = ctx.enter_context(tc.tile_pool(name="const", bufs=1))
        hpool = ctx.enter_context(tc.tile_pool(name="h", bufs=2))
        epool = ctx.enter_context(tc.tile_pool(name="evac", bufs=2))
        ipool = ctx.enter_context(tc.tile_pool(name="injd", bufs=2))
        ppool = ctx.enter_context(tc.tile_pool(name="psum", bufs=1, space="PSUM"))
        tpool = ctx.enter_context(tc.tile_pool(name="psumT", bufs=2, space="PSUM"))
        rpool = ctx.enter_context(tc.tile_pool(name="psumR", bufs=2, space="PSUM"))

        # resident W^T slabs: slab u = pass*16 + k-tile at cols [u*2048, ...).
        # Spread the 16 MB load across both HWDGE families + SWDGE.
        wt = const.tile([128, NSLAB * N], fmm)
        for u in range(NSLAB):
            eng = (nc.sync, nc.scalar, nc.gpsimd)[u % 3]
            eng.dma_start(wt[:, u * N : (u + 1) * N], wt_d[u * 128 : (u + 1) * 128, :])
        i128 = const.tile([128, BPC], fmm)
        nc.sync.dma_start(i128[:], i128_d[:])
        wsel = const.tile([128, 2 * NCHUNK], fmm)
        nc.sync.dma_start(wsel[:], wsel_d[:])
        outst = const.tile([2, n_steps * BPC], f32)

        psum = ppool.tile([128, N], f32)
        nc.vector.memset(psum[:], 0.0)

        hT = hpool.tile([128, NCHUNK * BPC], fmm)
        nc.vector.memset(hT[:], 0.0)

        tc.strict_bb_all_engine_barrier()

        # Tail work (transpose-sum + relu of banks 2-3, and the per-timestep
        # readout) is deferred into the NEXT microstep's instruction stream so
        # every PE wait lands >=2 banks after its producer. Legal because the
        # next microstep's round r only reads relu-bank r.
        pending = []

        for t in range(n_steps):
            injd = ipool.tile([128, 128], f32)
            nc.sync.dma_start(injd[:], injd_d[t])
            for s in range(4):
                # ---- main matmuls: psum[32j+b, n] += sum_k h[b,k] Wrec[n,k]
                # Bank-outer so bank n finishes early; its PSUM->SBUF evac and
                # transpose-sum matmuls then hide behind bank n+1's matmuls.
                # Within a bank, col-group j handles k-tiles {4r+j}; j
                # innermost so the 4 column-group streams overlap.
                evac = epool.tile([128, N], fmm)
                psumT = tpool.tile([128, NCHUNK * BPC], f32)

                def main_bank(n):
                    for r in range(4):
                        for p in range(npass):
                            for j in range(4):
                                kk = 4 * r + j
                                u = p * NCHUNK + kk
                                nc.tensor.matmul(
                                    psum[32 * j : 32 * j + BPC, 512 * n : 512 * (n + 1)],
                                    lhsT=hT[:, kk * BPC : (kk + 1) * BPC],
                                    rhs=wt[:, u * N + 512 * n : u * N + 512 * (n + 1)],
                                    start=(r == 0 and p == 0),
                                    stop=(r == 3 and p == npass - 1),
                                    tile_position=(0, 32 * j),
                                )

                def evac_bank(n):
                    # ACT copies cost ~2 us vs ~0.7 us on DVE; with 1-pass main
                    # matmuls the banks are too short to hide ACT, so keep all
                    # evacs on DVE there and alternate engines only for 2-pass.
                    if npass == 1 or n % 2 == 0:
                        nc.vector.tensor_copy(
                            evac[:, 512 * n : 512 * (n + 1)], psum[:, 512 * n : 512 * (n + 1)]
                        )
                    else:
                        nc.scalar.copy(
                            evac[:, 512 * n : 512 * (n + 1)], psum[:, 512 * n : 512 * (n + 1)]
                        )

                def tmm_bank(n, evac=evac, psumT=psumT):
                    # transpose-sum: psumT[m, c*8+b] = sum_j psum[32j+b, c*128+m]
                    for c in range(4 * n, 4 * n + 4):
                        nc.tensor.matmul(
                            psumT[:, c * BPC : (c + 1) * BPC],
                            lhsT=evac[:, c * 128 : (c + 1) * 128],
                            rhs=i128[:],
                            start=True,
                            stop=True,
                        )

                hT_new = hpool.tile([128, NCHUNK * BPC], fmm)

                def relu_bank(n, s=s, psumT=psumT, hT_new=hT_new, injd=injd):
                    # chunks 4n..4n+3 -> hT cols [32n, 32n+32); round r of the
                    # next microstep depends only on relu_bank(r).
                    cs = slice(32 * n, 32 * n + 32)
                    if s == 0:
                        nc.vector.tensor_add(hT_new[:, cs], psumT[:, cs], injd[:, cs])
                        nc.vector.tensor_relu(hT_new[:, cs], hT_new[:, cs])
                    else:
                        nc.vector.tensor_relu(hT_new[:, cs], psumT[:, cs])

                # flush deferred tail of the previous microstep first (its
                # relu banks 2-3 gate this microstep's rounds 2-3; its T-MMs
                # read an evac produced well before, so no PE wait).
                for fn in pending:
                    fn()
                pending = []

                main_bank(0)
                evac_bank(0)
                main_bank(1)
                evac_bank(1)
                tmm_bank(0)
                relu_bank(0)
                main_bank(2)
                evac_bank(2)
                tmm_bank(1)
                relu_bank(1)
                main_bank(3)
                evac_bank(3)
                pending = [
                    lambda n=2, f=tmm_bank: f(n),
                    lambda n=2, f=relu_bank: f(n),
                    lambda n=3, f=tmm_bank: f(n),
                    lambda n=3, f=relu_bank: f(n),
                ]
                hT = hT_new

            # ---- readout for timestep t from final hT (deferred behind the
            # pending relu banks 2-3 that complete that hT)
            def readout(t=t, hT=hT):
                pr = rpool.tile([2, BPC], f32)
                for c in range(NCHUNK):
                    nc.tensor.matmul(
                        pr[:],
                        lhsT=wsel[:, c * 2 : (c + 1) * 2],
                        rhs=hT[:, c * BPC : (c + 1) * BPC],
                        start=(c == 0),
                        stop=(c == NCHUNK - 1),
                    )
                nc.vector.tensor_copy(outst[:, t * BPC : (t + 1) * BPC], pr[:])

            pending.append(readout)

        for fn in pending:
            fn()
        nc.sync.dma_start(out_d[:], outst[:])
    nc.compile()
    return nc


def _prep_inputs(inputs, W_rec, W_in, b_in, W_out, sensory_indices, output_indices,
                 n_steps, mode=MODE):
    inputs = np.asarray(inputs, np.float32)
    W_rec = np.asarray(W_rec, np.float32)
    W_in = np.asarray(W_in, np.float32)
    b_in = np.asarray(b_in, np.float32)
    W_out = np.asarray(W_out, np.float32)
    sens = np.asarray(sensory_indices).astype(np.int64)
    oidx = np.asarray(output_indices).astype(np.int64)

    wtf = np.ascontiguousarray(W_rec.T)
    wsel_full = np.zeros((2, N), np.float32)
    np.add.at(wsel_full, (slice(None), oidx), W_out)
    wself = wsel_full.reshape(2, NCHUNK, 128).transpose(2, 1, 0).reshape(128, 2 * NCHUNK)

    if mode.startswith("fp16"):
        w1 = wtf.astype(np.float16)
        if mode == "fp16x2":
            w2 = (wtf - w1.astype(np.float32)).astype(np.float16)
            wt = np.ascontiguousarray(np.concatenate([w1, w2], axis=0))
        else:
            wt = np.ascontiguousarray(w1)
        wsel = np.ascontiguousarray(wself.astype(np.float16))
        i128 = (np.arange(128)[:, None] % 32 == np.arange(BPC)[None, :]).astype(np.float16)
    else:
        wt = wtf
        wsel = np.ascontiguousarray(wself)
        i128 = (np.arange(128)[:, None] % 32 == np.arange(BPC)[None, :]).astype(np.float32)

    # dense injection in hT layout, per core
    inj_all = inputs[:, :n_steps, :] @ W_in.T + b_in  # [B, T, 256]
    inj_dense = np.zeros((B, n_steps, N), np.float32)
    np.add.at(inj_dense, (slice(None), slice(None), sens), inj_all)
    injd_cores = []
    for g in range(NCORES):
        a = inj_dense[g * BPC : (g + 1) * BPC]  # [8, T, 2048]
        a = a.reshape(BPC, n_steps, NCHUNK, 128).transpose(1, 3, 2, 0)
        injd_cores.append(np.ascontiguousarray(a.reshape(n_steps, 128, NCHUNK * BPC)))

    return wt, injd_cores, wsel, i128


def _run(inputs, W_rec, W_in, b_in, W_out, b_out, sensory_indices, output_indices,
         K, n_steps=T, trace=False, mode=MODE):
    from concourse.bass_utils import run_bass_kernel_spmd

    assert int(K) == 4
    wt, injd_cores, wsel, i128 = _prep_inputs(
        inputs, W_rec, W_in, b_in, W_out, sensory_indices, output_indices,
        n_steps, mode)

    key = (n_steps, mode)
    if key not in _CACHE:
        _CACHE[key] = _build_nc(n_steps, mode)
    nc = _CACHE[key]

    in_maps = [
        {"wt": wt, "injd": injd_cores[g], "wsel": wsel, "i128": i128}
        for g in range(NCORES)
    ]
    res = run_bass_kernel_spmd(nc, in_maps, list(range(NCORES)), trace=trace)

    b_out = np.asarray(b_out, np.float32)
    outs = []
    for g in range(NCORES):
        r = np.asarray(res.results[g]["out"])  # [2, T*8]
        outs.append(r.reshape(2, n_steps, BPC).transpose(2, 1, 0))  # [8, T, 2]
    full = np.concatenate(outs, axis=0) + b_out  # [B, T, 2]
    return np.ascontiguousarray(full.astype(np.float32)), res


def kernel(**inputs):
    out, _ = _run(
        inputs["inputs"], inputs["W_rec"], inputs["W_in"], inputs["b_in"],
        inputs["W_out"], inputs["b_out"], inputs["sensory_indices"],
        inputs["output_indices"], inputs["K"],
    )
    return out


# revision 23
# speedup vs baseline: 1.8498x; 1.8493x over previous
"""Trainium2 Bass kernel for a dense recurrent scan (nn_CXBPU_55611236549128).

Math (per timestep t, K=4 microsteps):
    inj  = x_t @ W_in.T + b_in                  scattered into sensory_indices
    h    = relu(h @ W_rec.T + scatter(inj))     microstep 0
    h    = relu(h @ W_rec.T)                    microsteps 1..K-1
    out_t = h[:, output_indices] @ W_out.T + b_out

Sharding: data-parallel over batch, 8 rows per core, W_rec replicated.

Per-core design (feature-major "hT" layout [128 partitions, 16 chunks x 8 batch]):
  - W_rec.T resident in SBUF, streamed as the *moving* matmul operand every
    microstep (h-stationary keeps the weight transit on the fast streaming
    port instead of the 1.2 GHz LDWEIGHTS port).
  - Precision: W = W1 + W2 with both halves fp16 (exact 22-bit split; fp16
    subnormals are exact on the PE), h quantized to fp16 once per microstep
    by the relu write. Two fp16 passes accumulate in fp32 PSUM. End-to-end
    error vs fp32 reference ~4e-4 scale-relative absmax (the recurrence is
    contractive, spectral radius 0.9, so per-step quantization damps).
  - 4 k-tiles run concurrently in 4 PE column groups (tile_position=(0,32j)),
    issued column-group-innermost so the streams overlap.
  - A "transpose-sum" matmul against a 0/1 selector (i128) folds the 4
    partition groups back into feature-major hT for the next microstep
    (exact: fp16 values pass through fp32 PSUM untouched).
  - Injection is added as a host-precomputed dense tile already in hT layout.
  - Readout: 16 tiny matmuls vs scatter-expanded W_out (wsel).
"""

import os
from contextlib import ExitStack

import numpy as np

N = 2048
B = 64
T = 128
NCORES = 8
BPC = B // NCORES  # 8 batch rows per core
NCHUNK = N // 128  # 16

_CACHE = {}

# 'fp16x2' = two-pass fp16 split (fast), 'fp32' = exact fp32 (4-pass, slow)
MODE = os.environ.get("KERNEL_MM_MODE", "fp16x2")


def _build_nc(n_steps, mode=MODE):
    import concourse.bass as bass
    import concourse.mybir as mybir
    import concourse.tile as tile
    from concourse import bacc

    f32 = mybir.dt.float32
    f16 = mybir.dt.float16
    fmm = f16 if mode.startswith("fp16") else f32
    npass = 2 if mode == "fp16x2" else 1
    nc = bacc.Bacc(trn_type="TRN2")

    wt_d = nc.dram_tensor("wt", [npass * N, N], fmm, kind="ExternalInput")
    injd_d = nc.dram_tensor("injd", [n_steps, 128, 128], f32, kind="ExternalInput")
    wsel_d = nc.dram_tensor("wsel", [128, 2 * NCHUNK], fmm, kind="ExternalInput")
    i128_d = nc.dram_tensor("i128", [128, BPC], fmm, kind="ExternalInput")
    out_d = nc.dram_tensor("out", [2, n_steps * BPC], f32, kind="ExternalOutput")

    NSLAB = npass * NCHUNK

    with tile.TileContext(nc) as tc, ExitStack() as ctx:
        const = ctx.enter_context(tc.tile_pool(name="const", bufs=1))
        hpool = ctx.enter_context(tc.tile_pool(name="h", bufs=3))
        epool = ctx.enter_context(tc.tile_pool(name="evac", bufs=3))
        ipool = ctx.enter_context(tc.tile_pool(name="injd", bufs=2))
        ppool = ctx.enter_context(tc.tile_pool(name="psum", bufs=1, space="PSUM"))
        tpool = ctx.enter_context(tc.tile_pool(name="psumT", bufs=2, space="PSUM"))
        rpool = ctx.enter_context(tc.tile_pool(name="psumR", bufs=2, space="PSUM"))

        # resident W^T slabs: slab u = pass*16 + k-tile at cols [u*2048, ...).
        # Spread the 16 MB load across both HWDGE families + SWDGE.
        wt = const.tile([128, NSLAB * N], fmm)
        for u in range(NSLAB):
            eng = (nc.sync, nc.scalar, nc.gpsimd)[u % 3]
            eng.dma_start(wt[:, u * N : (u + 1) * N], wt_d[u * 128 : (u + 1) * 128, :])
        i128 = const.tile([128, BPC], fmm)
        nc.sync.dma_start(i128[:], i128_d[:])
        wsel = const.tile([128, 2 * NCHUNK], fmm)
        nc.sync.dma_start(wsel[:], wsel_d[:])
        outst = const.tile([2, n_steps * BPC], f32)

        psum = ppool.tile([128, N], f32)
        nc.vector.memset(psum[:], 0.0)

        hT = hpool.tile([128, NCHUNK * BPC], fmm)
        nc.vector.memset(hT[:], 0.0)

        tc.strict_bb_all_engine_barrier()

        # Tail work (transpose-sum + relu of banks 2-3, and the per-timestep
        # readout) is deferred into the NEXT microstep's instruction stream so
        # every PE wait lands >=2 banks after its producer. Legal because the
        # next microstep's round r only reads relu-bank r.
        pending = []

        for t in range(n_steps):
            injd = ipool.tile([128, 128], f32)
            nc.sync.dma_start(injd[:], injd_d[t])
            for s in range(4):
                # ---- main matmuls: psum[32j+b, n] += sum_k h[b,k] Wrec[n,k]
                # Bank-outer so bank n finishes early; its PSUM->SBUF evac and
                # transpose-sum matmuls then hide behind bank n+1's matmuls.
                # Within a bank, col-group j handles k-tiles {4r+j}; j
                # innermost so the 4 column-group streams overlap.
                evac = epool.tile([128, N], fmm)
                psumT = tpool.tile([128, NCHUNK * BPC], f32)

                def main_bank(n):
                    for r in range(4):
                        for p in range(npass):
                            for j in range(4):
                                kk = 4 * r + j
                                u = p * NCHUNK + kk
                                nc.tensor.matmul(
                                    psum[32 * j : 32 * j + BPC, 512 * n : 512 * (n + 1)],
                                    lhsT=hT[:, kk * BPC : (kk + 1) * BPC],
                                    rhs=wt[:, u * N + 512 * n : u * N + 512 * (n + 1)],
                                    start=(r == 0 and p == 0),
                                    stop=(r == 3 and p == npass - 1),
                                    tile_position=(0, 32 * j),
                                )

                def evac_bank(n):
                    # ACT copies cost ~2 us vs ~0.7 us on DVE; with 1-pass main
                    # matmuls the banks are too short to hide ACT, so keep all
                    # evacs on DVE there and alternate engines only for 2-pass.
                    if npass == 1 or n % 2 == 0:
                        nc.vector.tensor_copy(
                            evac[:, 512 * n : 512 * (n + 1)], psum[:, 512 * n : 512 * (n + 1)]
                        )
                    else:
                        nc.scalar.copy(
                            evac[:, 512 * n : 512 * (n + 1)], psum[:, 512 * n : 512 * (n + 1)]
                        )

                def tmm_bank(n, evac=evac, psumT=psumT):
                    # transpose-sum: psumT[m, c*8+b] = sum_j psum[32j+b, c*128+m]
                    for c in range(4 * n, 4 * n + 4):
                        nc.tensor.matmul(
                            psumT[:, c * BPC : (c + 1) * BPC],
                            lhsT=evac[:, c * 128 : (c + 1) * 128],
                            rhs=i128[:],
                            start=True,
                            stop=True,
                        )

                hT_new = hpool.tile([128, NCHUNK * BPC], fmm)

                def relu_bank(n, s=s, psumT=psumT, hT_new=hT_new, injd=injd):
                    # chunks 4n..4n+3 -> hT cols [32n, 32n+32); round r of the
                    # next microstep depends only on relu_bank(r).
                    cs = slice(32 * n, 32 * n + 32)
                    if s == 0:
                        nc.vector.tensor_add(hT_new[:, cs], psumT[:, cs], injd[:, cs])
                        nc.vector.tensor_relu(hT_new[:, cs], hT_new[:, cs])
                    else:
                        nc.vector.tensor_relu(hT_new[:, cs], psumT[:, cs])

                # flush deferred tail of the previous microstep first (its
                # relu banks 2-3 gate this microstep's rounds 2-3; its T-MMs
                # read an evac produced well before, so no PE wait).
                for fn in pending:
                    fn()
                pending = []

                main_bank(0)
                evac_bank(0)
                main_bank(1)
                evac_bank(1)
                tmm_bank(0)
                relu_bank(0)
                main_bank(2)
                evac_bank(2)
                tmm_bank(1)
                relu_bank(1)
                main_bank(3)
                evac_bank(3)
                pending = [
                    lambda n=2, f=tmm_bank: f(n),
                    lambda n=2, f=relu_bank: f(n),
                    lambda n=3, f=tmm_bank: f(n),
                    lambda n=3, f=relu_bank: f(n),
                ]
                hT = hT_new

            # ---- readout for timestep t from final hT (deferred behind the
            # pending relu banks 2-3 that complete that hT)
            def readout(t=t, hT=hT):
                pr = rpool.tile([2, BPC], f32)
                for c in range(NCHUNK):
                    nc.tensor.matmul(
                        pr[:],
                        lhsT=wsel[:, c * 2 : (c + 1) * 2],
                        rhs=hT[:, c * BPC : (c + 1) * BPC],
                        start=(c == 0),
                        stop=(c == NCHUNK - 1),
                    )
                nc.vector.tensor_copy(outst[:, t * BPC : (t + 1) * BPC], pr[:])

            pending.append(readout)

        for fn in pending:
            fn()
        nc.sync.dma_start(out_d[:], outst[:])
    nc.compile()
    return nc


def _prep_inputs(inputs, W_rec, W_in, b_in, W_out, sensory_indices, output_indices,
                 n_steps, mode=MODE):
    inputs = np.asarray(inputs, np.float32)
    W_rec = np.asarray(W_rec, np.float32)
    W_in = np.asarray(W_in, np.float32)
    b_in = np.asarray(b_in, np.float32)
    W_out = np.asarray(W_out, np.float32)
    sens = np.asarray(sensory_indices).astype(np.int64)
    oidx = np.asarray(output_indices).astype(np.int64)

    wtf = np.ascontiguousarray(W_rec.T)
    wsel_full = np.zeros((2, N), np.float32)
    np.add.at(wsel_full, (slice(None), oidx), W_out)
    wself = wsel_full.reshape(2, NCHUNK, 128).transpose(2, 1, 0).reshape(128, 2 * NCHUNK)

    if mode.startswith("fp16"):
        w1 = wtf.astype(np.float16)
        if mode == "fp16x2":
            w2 = (wtf - w1.astype(np.float32)).astype(np.float16)
            wt = np.ascontiguousarray(np.concatenate([w1, w2], axis=0))
        else:
            wt = np.ascontiguousarray(w1)
        wsel = np.ascontiguousarray(wself.astype(np.float16))
        i128 = (np.arange(128)[:, None] % 32 == np.arange(BPC)[None, :]).astype(np.float16)
    else:
        wt = wtf
        wsel = np.ascontiguousarray(wself)
        i128 = (np.arange(128)[:, None] % 32 == np.arange(BPC)[None, :]).astype(np.float32)

    # dense injection in hT layout, per core
    inj_all = inputs[:, :n_steps, :] @ W_in.T + b_in  # [B, T, 256]
    inj_dense = np.zeros((B, n_steps, N), np.float32)
    np.add.at(inj_dense, (slice(None), slice(None), sens), inj_all)
    injd_cores = []
    for g in range(NCORES):
        a = inj_dense[g * BPC : (g + 1) * BPC]  # [8, T, 2048]
        a = a.reshape(BPC, n_steps, NCHUNK, 128).transpose(1, 3, 2, 0)
        injd_cores.append(np.ascontiguousarray(a.reshape(n_steps, 128, NCHUNK * BPC)))

    return wt, injd_cores, wsel, i128


def _run(inputs, W_rec, W_in, b_in, W_out, b_out, sensory_indices, output_indices,
         K, n_steps=T, trace=False, mode=MODE):
    from concourse.bass_utils import run_bass_kernel_spmd

    assert int(K) == 4
    wt, injd_cores, wsel, i128 = _prep_inputs(
        inputs, W_rec, W_in, b_in, W_out, sensory_indices, output_indices,
        n_steps, mode)

    key = (n_steps, mode)
    if key not in _CACHE:
        _CACHE[key] = _build_nc(n_steps, mode)
    nc = _CACHE[key]

    in_maps = [
        {"wt": wt, "injd": injd_cores[g], "wsel": wsel, "i128": i128}
        for g in range(NCORES)
    ]
    res = run_bass_kernel_spmd(nc, in_maps, list(range(NCORES)), trace=trace)

    b_out = np.asarray(b_out, np.float32)
    outs = []
    for g in range(NCORES):
        r = np.asarray(res.results[g]["out"])  # [2, T*8]
        outs.append(r.reshape(2, n_steps, BPC).transpose(2, 1, 0))  # [8, T, 2]
    full = np.concatenate(outs, axis=0) + b_out  # [B, T, 2]
    return np.ascontiguousarray(full.astype(np.float32)), res


def kernel(**inputs):
    out, _ = _run(
        inputs["inputs"], inputs["W_rec"], inputs["W_in"], inputs["b_in"],
        inputs["W_out"], inputs["b_out"], inputs["sensory_indices"],
        inputs["output_indices"], inputs["K"],
    )
    return out


# revision 24
# speedup vs baseline: 1.8516x; 1.0010x over previous
"""Trainium2 Bass kernel for a dense recurrent scan (nn_CXBPU_55611236549128).

Math (per timestep t, K=4 microsteps):
    inj  = x_t @ W_in.T + b_in                  scattered into sensory_indices
    h    = relu(h @ W_rec.T + scatter(inj))     microstep 0
    h    = relu(h @ W_rec.T)                    microsteps 1..K-1
    out_t = h[:, output_indices] @ W_out.T + b_out

Sharding: data-parallel over batch, 8 rows per core, W_rec replicated.

Per-core design (feature-major "hT" layout [128 partitions, 16 chunks x 8 batch]):
  - W_rec.T resident in SBUF, streamed as the *moving* matmul operand every
    microstep (h-stationary keeps the weight transit on the fast streaming
    port instead of the 1.2 GHz LDWEIGHTS port).
  - Precision: W = W1 + W2 with both halves fp16 (exact 22-bit split; fp16
    subnormals are exact on the PE), h quantized to fp16 once per microstep
    by the relu write. Two fp16 passes accumulate in fp32 PSUM. End-to-end
    error vs fp32 reference ~4e-4 scale-relative absmax (the recurrence is
    contractive, spectral radius 0.9, so per-step quantization damps).
  - 4 k-tiles run concurrently in 4 PE column groups (tile_position=(0,32j)),
    issued column-group-innermost so the streams overlap.
  - A "transpose-sum" matmul against a 0/1 selector (i128) folds the 4
    partition groups back into feature-major hT for the next microstep
    (exact: fp16 values pass through fp32 PSUM untouched).
  - Injection is added as a host-precomputed dense tile already in hT layout.
  - Readout: 16 tiny matmuls vs scatter-expanded W_out (wsel).
"""

import os
from contextlib import ExitStack

import numpy as np

N = 2048
B = 64
T = 128
NCORES = 8
BPC = B // NCORES  # 8 batch rows per core
NCHUNK = N // 128  # 16

_CACHE = {}

# 'fp16x2' = two-pass fp16 split (fast), 'fp32' = exact fp32 (4-pass, slow)
MODE = os.environ.get("KERNEL_MM_MODE", "fp16x2")


def _build_nc(n_steps, mode=MODE):
    import concourse.bass as bass
    import concourse.mybir as mybir
    import concourse.tile as tile
    from concourse import bacc

    f32 = mybir.dt.float32
    f16 = mybir.dt.float16
    fmm = f16 if mode.startswith("fp16") else f32
    npass = 2 if mode == "fp16x2" else 1
    nc = bacc.Bacc(trn_type="TRN2")

    wt_d = nc.dram_tensor("wt", [npass * N, N], fmm, kind="ExternalInput")
    injd_d = nc.dram_tensor("injd", [n_steps, 128, 128], f32, kind="ExternalInput")
    wsel_d = nc.dram_tensor("wsel", [128, 2 * NCHUNK], fmm, kind="ExternalInput")
    i128_d = nc.dram_tensor("i128", [128, BPC], fmm, kind="ExternalInput")
    out_d = nc.dram_tensor("out", [2, n_steps * BPC], f32, kind="ExternalOutput")

    NSLAB = npass * NCHUNK

    with tile.TileContext(nc) as tc, ExitStack() as ctx:
        const = ctx.enter_context(tc.tile_pool(name="const", bufs=1))
        hpool = ctx.enter_context(tc.tile_pool(name="h", bufs=2))
        epool = ctx.enter_context(tc.tile_pool(name="evac", bufs=2))
        ipool = ctx.enter_context(tc.tile_pool(name="injd", bufs=2))
        ppool = ctx.enter_context(tc.tile_pool(name="psum", bufs=1, space="PSUM"))
        tpool = ctx.enter_context(tc.tile_pool(name="psumT", bufs=2, space="PSUM"))
        rpool = ctx.enter_context(tc.tile_pool(name="psumR", bufs=2, space="PSUM"))

        # resident W^T slabs: slab u = pass*16 + k-tile at cols [u*2048, ...).
        # Spread the 16 MB load across both HWDGE families + SWDGE.
        wt = const.tile([128, NSLAB * N], fmm)
        for u in range(NSLAB):
            eng = (nc.sync, nc.scalar, nc.gpsimd)[u % 3]
            eng.dma_start(wt[:, u * N : (u + 1) * N], wt_d[u * 128 : (u + 1) * 128, :])
        i128 = const.tile([128, BPC], fmm)
        nc.sync.dma_start(i128[:], i128_d[:])
        wsel = const.tile([128, 2 * NCHUNK], fmm)
        nc.sync.dma_start(wsel[:], wsel_d[:])
        outst = const.tile([2, n_steps * BPC], f32)

        psum = ppool.tile([128, N], f32)
        nc.vector.memset(psum[:], 0.0)

        hT = hpool.tile([128, NCHUNK * BPC], fmm)
        nc.vector.memset(hT[:], 0.0)

        tc.strict_bb_all_engine_barrier()

        # Tail work (transpose-sum + relu of banks 2-3, and the per-timestep
        # readout) is deferred into the NEXT microstep's instruction stream so
        # every PE wait lands >=2 banks after its producer. Legal because the
        # next microstep's round r only reads relu-bank r.
        pending = []

        for t in range(n_steps):
            injd = ipool.tile([128, 128], f32)
            nc.sync.dma_start(injd[:], injd_d[t])
            for s in range(4):
                # ---- main matmuls: psum[32j+b, n] += sum_k h[b,k] Wrec[n,k]
                # Bank-outer so bank n finishes early; its PSUM->SBUF evac and
                # transpose-sum matmuls then hide behind bank n+1's matmuls.
                # Within a bank, col-group j handles k-tiles {4r+j}; j
                # innermost so the 4 column-group streams overlap.
                evac = epool.tile([128, N], fmm)
                psumT = tpool.tile([128, NCHUNK * BPC], f32)

                def main_bank(n):
                    for r in range(4):
                        for p in range(npass):
                            for j in range(4):
                                kk = 4 * r + j
                                u = p * NCHUNK + kk
                                nc.tensor.matmul(
                                    psum[32 * j : 32 * j + BPC, 512 * n : 512 * (n + 1)],
                                    lhsT=hT[:, kk * BPC : (kk + 1) * BPC],
                                    rhs=wt[:, u * N + 512 * n : u * N + 512 * (n + 1)],
                                    start=(r == 0 and p == 0),
                                    stop=(r == 3 and p == npass - 1),
                                    tile_position=(0, 32 * j),
                                )

                def evac_bank(n):
                    # ACT copies cost ~2 us vs ~0.7 us on DVE; with 1-pass main
                    # matmuls the banks are too short to hide ACT, so keep all
                    # evacs on DVE there and alternate engines only for 2-pass.
                    if npass == 1 or n % 2 == 0:
                        nc.vector.tensor_copy(
                            evac[:, 512 * n : 512 * (n + 1)], psum[:, 512 * n : 512 * (n + 1)]
                        )
                    else:
                        nc.scalar.copy(
                            evac[:, 512 * n : 512 * (n + 1)], psum[:, 512 * n : 512 * (n + 1)]
                        )

                def tmm_bank(n, evac=evac, psumT=psumT):
                    # transpose-sum: psumT[m, c*8+b] = sum_j psum[32j+b, c*128+m]
                    for c in range(4 * n, 4 * n + 4):
                        nc.tensor.matmul(
                            psumT[:, c * BPC : (c + 1) * BPC],
                            lhsT=evac[:, c * 128 : (c + 1) * 128],
                            rhs=i128[:],
                            start=True,
                            stop=True,
                        )

                hT_new = hpool.tile([128, NCHUNK * BPC], fmm)

                def relu_bank(n, s=s, psumT=psumT, hT_new=hT_new, injd=injd):
                    # chunks 4n..4n+3 -> hT cols [32n, 32n+32); round r of the
                    # next microstep depends only on relu_bank(r).
                    cs = slice(32 * n, 32 * n + 32)
                    if s == 0:
                        nc.vector.tensor_add(hT_new[:, cs], psumT[:, cs], injd[:, cs])
                        nc.vector.tensor_relu(hT_new[:, cs], hT_new[:, cs])
                    else:
                        nc.vector.tensor_relu(hT_new[:, cs], psumT[:, cs])

                # flush deferred tail of the previous microstep first (its
                # relu banks 2-3 gate this microstep's rounds 2-3; its T-MMs
                # read an evac produced well before, so no PE wait).
                for fn in pending:
                    fn()
                pending = []

                main_bank(0)
                evac_bank(0)
                main_bank(1)
                evac_bank(1)
                tmm_bank(0)
                relu_bank(0)
                main_bank(2)
                evac_bank(2)
                tmm_bank(1)
                relu_bank(1)
                main_bank(3)
                evac_bank(3)
                pending = [
                    lambda n=2, f=tmm_bank: f(n),
                    lambda n=2, f=relu_bank: f(n),
                    lambda n=3, f=tmm_bank: f(n),
                    lambda n=3, f=relu_bank: f(n),
                ]
                hT = hT_new

            # ---- readout for timestep t from final hT (deferred behind the
            # pending relu banks 2-3 that complete that hT)
            def readout(t=t, hT=hT):
                pr = rpool.tile([2, BPC], f32)
                for c in range(NCHUNK):
                    nc.tensor.matmul(
                        pr[:],
                        lhsT=wsel[:, c * 2 : (c + 1) * 2],
                        rhs=hT[:, c * BPC : (c + 1) * BPC],
                        start=(c == 0),
                        stop=(c == NCHUNK - 1),
                    )
                nc.vector.tensor_copy(outst[:, t * BPC : (t + 1) * BPC], pr[:])

            pending.append(readout)

        for fn in pending:
            fn()
        nc.sync.dma_start(out_d[:], outst[:])
    nc.compile()
    return nc


def _prep_inputs(inputs, W_rec, W_in, b_in, W_out, sensory_indices, output_indices,
                 n_steps, mode=MODE):
    inputs = np.asarray(inputs, np.float32)
    W_rec = np.asarray(W_rec, np.float32)
    W_in = np.asarray(W_in, np.float32)
    b_in = np.asarray(b_in, np.float32)
    W_out = np.asarray(W_out, np.float32)
    sens = np.asarray(sensory_indices).astype(np.int64)
    oidx = np.asarray(output_indices).astype(np.int64)

    wtf = np.ascontiguousarray(W_rec.T)
    wsel_full = np.zeros((2, N), np.float32)
    np.add.at(wsel_full, (slice(None), oidx), W_out)
    wself = wsel_full.reshape(2, NCHUNK, 128).transpose(2, 1, 0).reshape(128, 2 * NCHUNK)

    if mode.startswith("fp16"):
        w1 = wtf.astype(np.float16)
        if mode == "fp16x2":
            w2 = (wtf - w1.astype(np.float32)).astype(np.float16)
            wt = np.ascontiguousarray(np.concatenate([w1, w2], axis=0))
        else:
            wt = np.ascontiguousarray(w1)
        wsel = np.ascontiguousarray(wself.astype(np.float16))
        i128 = (np.arange(128)[:, None] % 32 == np.arange(BPC)[None, :]).astype(np.float16)
    else:
        wt = wtf
        wsel = np.ascontiguousarray(wself)
        i128 = (np.arange(128)[:, None] % 32 == np.arange(BPC)[None, :]).astype(np.float32)

    # dense injection in hT layout, per core
    inj_all = inputs[:, :n_steps, :] @ W_in.T + b_in  # [B, T, 256]
    inj_dense = np.zeros((B, n_steps, N), np.float32)
    np.add.at(inj_dense, (slice(None), slice(None), sens), inj_all)
    injd_cores = []
    for g in range(NCORES):
        a = inj_dense[g * BPC : (g + 1) * BPC]  # [8, T, 2048]
        a = a.reshape(BPC, n_steps, NCHUNK, 128).transpose(1, 3, 2, 0)
        injd_cores.append(np.ascontiguousarray(a.reshape(n_steps, 128, NCHUNK * BPC)))

    return wt, injd_cores, wsel, i128


def _run(inputs, W_rec, W_in, b_in, W_out, b_out, sensory_indices, output_indices,
         K, n_steps=T, trace=False, mode=MODE):
    from concourse.bass_utils import run_bass_kernel_spmd

    assert int(K) == 4
    wt, injd_cores, wsel, i128 = _prep_inputs(
        inputs, W_rec, W_in, b_in, W_out, sensory_indices, output_indices,
        n_steps, mode)

    key = (n_steps, mode)
    if key not in _CACHE:
        _CACHE[key] = _build_nc(n_steps, mode)
    nc = _CACHE[key]

    in_maps = [
        {"wt": wt, "injd": injd_cores[g], "wsel": wsel, "i128": i128}
        for g in range(NCORES)
    ]
    res = run_bass_kernel_spmd(nc, in_maps, list(range(NCORES)), trace=trace)

    b_out = np.asarray(b_out, np.float32)
    outs = []
    for g in range(NCORES):
        r = np.asarray(res.results[g]["out"])  # [2, T*8]
        outs.append(r.reshape(2, n_steps, BPC).transpose(2, 1, 0))  # [8, T, 2]
    full = np.concatenate(outs, axis=0) + b_out  # [B, T, 2]
    return np.ascontiguousarray(full.astype(np.float32)), res


def kernel(**inputs):
    out, _ = _run(
        inputs["inputs"], inputs["W_rec"], inputs["W_in"], inputs["b_in"],
        inputs["W_out"], inputs["b_out"], inputs["sensory_indices"],
        inputs["output_indices"], inputs["K"],
    )
    return out
